# revision 1
# baseline (speedup 1.0000x reference)
# Trainium2 Bass kernel for nn_CausalSelfAttention_13022340841799.
#
# Problem (hardcoded shapes): B=2, L=4096, D=512, 8 heads of dim 64.
#   qkv = x @ w_in + b_in; prefix-causal attention (PREFIX=1: tril mask with
#   column 0 disallowed for rows >= 1); out = attn_out @ w_out + b_out.
#
# Sharding: 8 cores = 2 batches x 4 head-pairs. Core c handles batch c//4 and
# heads {2*(c%4), 2*(c%4)+1}. Each core computes a partial [L, D] output
# (its heads' contribution through w_out); the host sums the 4 partials per
# batch and adds b_out.
#
# Device algorithm (per core), flash-attention style in "transposed" layout:
#   xT [D, L] built via PE transposes; qT/kT = (w.T @ xT) [128, L];
#   v natural [L, 128]. Per head: S^T tiles [128 cols, 512 rows] = k_j^T q_r,
#   P^T = exp(S^T/8) * mask (bf16), O_aug^T [65, 512] += v_aug_j^T @ P^T where
#   v_aug has a ones column => row 64 accumulates the softmax denominator.
#   Normalize with DVE reciprocal + gpsimd partition broadcast, then
#   out partial = sum_h O_h @ w_out[h rows] in PSUM, DMA to DRAM.
# Compute dtype bf16 (f32 PSUM accumulate); masks/causal-skip halve the work.

import numpy as np

import concourse.bass as bass
import concourse.mybir as mybir
import concourse.tile as tile
from concourse import bacc
from concourse.bass_utils import run_bass_kernel_spmd
from concourse.masks import make_identity

F32 = mybir.dt.float32
BF16 = mybir.dt.bfloat16

B, L, D = 2, 4096, 512
H, HD = 8, 64
HPC = 2                  # heads per core
CD = HPC * HD            # 128 per-core qkv feature columns
NCORES = 8
SCALE = 1.0 / 8.0        # 1/sqrt(64)
RB = L // 128            # 32 row blocks
NRC = L // 512           # 8 row chunks
KC = D // 128            # 4 contraction chunks


def _build_masks(nc, pool):
    """Six [128, 512] bf16 {0,1} mask tiles for S^T tiles [c=col, rr=row].

    d0..d3: diagonal-range masks, allowed iff 128*d + c <= rr.
    j0r0:   col block 0, row chunk 0: (c <= rr) and (c >= 1 or rr == 0).
    j0:     col block 0, row chunk > 0: c >= 1.
    """
    masks = {}
    for d in range(4):
        m = pool.tile([128, 512], BF16, name=f"mask_d{d}")
        nc.gpsimd.memset(m, 1.0)
        # value = base + cm*partition + step*free ; keep where >= 0 else fill
        nc.gpsimd.affine_select(
            out=m, in_=m, compare_op=mybir.AluOpType.is_ge, fill=0.0,
            base=-128 * d, channel_multiplier=-1, pattern=[[1, 512]],
        )
        masks[f"d{d}"] = m
    j0r0 = pool.tile([128, 512], BF16, name="mask_j0r0")
    nc.gpsimd.memset(j0r0, 1.0)
    nc.gpsimd.affine_select(
        out=j0r0, in_=j0r0, compare_op=mybir.AluOpType.is_ge, fill=0.0,
        base=0, channel_multiplier=-1, pattern=[[1, 512]],
    )
    nc.gpsimd.memset(j0r0[0:1, 1:512], 0.0)   # row 0 of cols: col0 only for row 0
    masks["j0r0"] = j0r0
    j0 = pool.tile([128, 512], BF16, name="mask_j0")
    nc.gpsimd.memset(j0, 1.0)
    nc.gpsimd.memset(j0[0:1, :], 0.0)          # col 0 masked for all rows >= 1
    masks["j0"] = j0
    return masks


def _mask_for(masks, r, j):
    if j == 0:
        return masks["j0r0"] if r == 0 else masks["j0"]
    d = j - 4 * r
    if 0 <= d <= 3:
        return masks[f"d{d}"]
    return None


def build_kernel(dbg_stage="full"):
    nc = bacc.Bacc(trn_type="TRN2", target_bir_lowering=False)

    x_d = nc.declare_dram_parameter("x", [L, D], F32, isOutput=False)
    wq_d = nc.declare_dram_parameter("wq", [D, CD], F32, isOutput=False)
    wk_d = nc.declare_dram_parameter("wk", [D, CD], F32, isOutput=False)
    wv_d = nc.declare_dram_parameter("wv", [D, CD], F32, isOutput=False)
    wo_d = nc.declare_dram_parameter("wo", [CD, D], F32, isOutput=False)
    bq_d = nc.declare_dram_parameter("bq", [CD], F32, isOutput=False)
    bk_d = nc.declare_dram_parameter("bk", [CD], F32, isOutput=False)
    bv_d = nc.declare_dram_parameter("bv", [CD], F32, isOutput=False)
    out_d = nc.declare_dram_parameter("out", [L, D], F32, isOutput=True)

    with tile.TileContext(nc) as tc:
        with (
            tc.tile_pool(name="const", bufs=1) as const,
            tc.tile_pool(name="stage", bufs=3) as stage,
            tc.tile_pool(name="work", bufs=4) as work,
        ):
            # ---- constants / static tensors
            ident = const.tile([128, 128], BF16, name="ident")
            make_identity(nc, ident)
            masks = _build_masks(nc, const)
            ones_k1 = const.tile([1, CD], BF16, name="ones_k1")
            nc.vector.memset(ones_k1, 1.0)

            xT = const.tile([128, KC, L], BF16, name="xT")        # [D-chunk, d, L]
            qT = const.tile([128, L], BF16, name="qT")            # 2 heads stacked
            kT = const.tile([128, L], BF16, name="kT")
            v0 = const.tile([128, RB, 65], BF16, name="v0")       # v_aug per col block
            v1 = const.tile([128, RB, 65], BF16, name="v1")
            nc.vector.memset(v0[:, :, 64:65], 1.0)
            nc.vector.memset(v1[:, :, 64:65], 1.0)
            O_all = const.tile([64, HPC, L], BF16, name="O_all")  # normalized attn out^T

            # ---- weights: load f32, cast to bf16 in matmul layouts
            wq_f = const.tile([128, KC, CD], F32, name="wq_f")
            nc.sync.dma_start(wq_f, wq_d.rearrange("(o p) c -> p o c", p=128))
            wq_b = const.tile([128, KC, CD], BF16, name="wq_b")
            nc.vector.tensor_copy(wq_b, wq_f)

            wk_f = const.tile([128, KC, CD], F32, name="wk_f")
            nc.sync.dma_start(wk_f, wk_d.rearrange("(o p) c -> p o c", p=128))
            wk_b = const.tile([128, KC, CD], BF16, name="wk_b")
            nc.vector.tensor_copy(wk_b, wk_f)

            wv_f = const.tile([128, KC, CD], F32, name="wv_f")
            nc.sync.dma_start(wv_f, wv_d.rearrange("(o p) c -> p o c", p=128))
            wv_b = const.tile([128, KC, CD], BF16, name="wv_b")
            nc.vector.tensor_copy(wv_b, wv_f)

            # wo: [128, 512] -> [64 rows, 2 heads, 512] (head on free dim, lane aligned)
            wo_f = const.tile([64, HPC, D], F32, name="wo_f")
            nc.sync.dma_start(wo_f, wo_d.rearrange("(h r) n -> r h n", h=HPC))
            wo_b = const.tile([64, HPC, D], BF16, name="wo_b")
            nc.vector.tensor_copy(wo_b, wo_f)

            bq_s = const.tile([CD, 1], F32, name="bq_s")
            nc.sync.dma_start(bq_s, bq_d.rearrange("(p o) -> p o", o=1))
            bk_s = const.tile([CD, 1], F32, name="bk_s")
            nc.sync.dma_start(bk_s, bk_d.rearrange("(p o) -> p o", o=1))
            bv_f = const.tile([1, CD], F32, name="bv_f")
            nc.sync.dma_start(bv_f, bv_d.rearrange("(o c) -> o c", o=1))
            bv_b = const.tile([1, CD], BF16, name="bv_b")
            nc.vector.tensor_copy(bv_b, bv_f)

            # ---- phase A: xT via PE transpose; phase B: qkv projections
            with tc.tile_pool(name="psAB", bufs=2, space="PSUM") as psAB:
                for rb in range(RB):
                    xf = stage.tile([128, D], F32, tag="xf")
                    nc.sync.dma_start(xf, x_d[rb * 128:(rb + 1) * 128, :])
                    xb = stage.tile([128, D], BF16, tag="xb")
                    nc.vector.tensor_copy(xb, xf)
                    for d in range(KC):
                        pt = psAB.tile([128, 128], BF16, tag="pt", bufs=3)
                        nc.tensor.transpose(pt, xb[:, d * 128:(d + 1) * 128], ident)
                        nc.any.tensor_copy(xT[:, d, rb * 128:(rb + 1) * 128], pt)

                for nb in range(L // 512):
                    ns = slice(nb * 512, (nb + 1) * 512)
                    for wt, bt, dstT in ((wq_b, bq_s, qT), (wk_b, bk_s, kT)):
                        pq = psAB.tile([128, 512], F32, tag="pq", bufs=2)
                        for d in range(KC):
                            nc.tensor.matmul(
                                pq, lhsT=wt[:, d, :], rhs=xT[:, d, ns],
                                start=(d == 0), stop=(d == KC - 1),
                            )
                        nc.vector.tensor_scalar_add(dstT[:, ns], pq, bt)

                for rb in range(RB):
                    rs = slice(rb * 128, (rb + 1) * 128)
                    pv = psAB.tile([128, 512], F32, tag="pq", bufs=2)
                    for d in range(KC):
                        nc.tensor.matmul(
                            pv[:, :CD], lhsT=xT[:, d, rs], rhs=wv_b[:, d, :],
                            start=(d == 0), stop=False,
                        )
                    nc.tensor.matmul(
                        pv[:, :CD], lhsT=ones_k1, rhs=bv_b, start=False, stop=True,
                    )
                    nc.any.tensor_copy(v0[:, rb, 0:64], pv[:, 0:64])
                    nc.any.tensor_copy(v1[:, rb, 0:64], pv[:, 64:128])

            # ---- phase C: attention per head; phase D: output projection
            with (
                tc.tile_pool(name="psC", bufs=1, space="PSUM") as psC,
                tc.tile_pool(name="psD", bufs=3, space="PSUM") as psD,
                tc.tile_pool(name="dramp", bufs=3, space="DRAM") as dramp,
            ):
                for h in range(HPC):
                    hs = slice(h * 64, (h + 1) * 64)
                    vh = v0 if h == 0 else v1
                    for r in range(NRC):
                        rs = slice(r * 512, (r + 1) * 512)
                        po_t = psC.tile([65, 512], F32, tag="po", bufs=2)
                        njb = 4 * r + 4
                        for j in range(njb):
                            ss = psC.tile([128, 512], F32, tag="ss", bufs=3)
                            nc.tensor.matmul(
                                ss, lhsT=kT[hs, j * 128:(j + 1) * 128],
                                rhs=qT[hs, rs], start=True, stop=True,
                            )
                            p_sb = work.tile([128, 512], BF16, tag="p_sb")
                            nc.scalar.activation(
                                p_sb, ss, mybir.ActivationFunctionType.Exp,
                                scale=SCALE,
                            )
                            m = _mask_for(masks, r, j)
                            if m is not None:
                                nc.vector.tensor_mul(out=p_sb, in0=p_sb, in1=m)
                            nc.tensor.matmul(
                                po_t, lhsT=vh[:, j, :], rhs=p_sb,
                                start=(j == 0), stop=(j == njb - 1),
                            )
                        rr_t = work.tile([65, 512], F32, tag="rr")
                        nc.vector.reciprocal(rr_t[64:65, :], po_t[64:65, :])
                        # broadcast partition 64 -> 0..63 via DRAM bounce
                        # (gpsimd partition_broadcast crashes the exec unit on HW)
                        scr = dramp.tile([1, 512], F32, tag="scr")
                        nc.sync.dma_start(out=scr[0:1, :], in_=rr_t[64:65, :])
                        s = scr[0:1, :]
                        src_b = bass.AP(
                            tensor=s.tensor, offset=s.offset,
                            ap=[[0, 64]] + [list(p) for p in s.ap[1:]],
                        )
                        nc.sync.dma_start(out=rr_t[0:64, :], in_=src_b)
                        nc.vector.tensor_tensor(
                            O_all[:, h, rs], po_t[0:64, :], rr_t[0:64, :],
                            mybir.AluOpType.mult,
                        )

                for rb in range(RB):
                    rs = slice(rb * 128, (rb + 1) * 128)
                    pod = psD.tile([128, 512], F32, tag="pod", bufs=3)
                    for h in range(HPC):
                        nc.tensor.matmul(
                            pod, lhsT=O_all[:, h, rs], rhs=wo_b[:, h, :],
                            start=(h == 0), stop=(h == HPC - 1),
                        )
                    ot = stage.tile([128, D], F32, tag="ot")
                    nc.any.tensor_copy(ot, pod)
                    nc.sync.dma_start(out_d[rs, :], ot)

    nc.finalize()
    return nc


def _shard_inputs(x, w_in, b_in, w_out):
    """Per-core input maps: core c -> batch c//4, heads pair c%4."""
    in_maps = []
    for c in range(NCORES):
        b = c // 4
        hp = c % 4
        cs = slice(hp * CD, hp * CD + CD)
        in_maps.append({
            "x": np.ascontiguousarray(x[b]),
            "wq": np.ascontiguousarray(w_in[:, 0:D][:, cs]),
            "wk": np.ascontiguousarray(w_in[:, D:2 * D][:, cs]),
            "wv": np.ascontiguousarray(w_in[:, 2 * D:3 * D][:, cs]),
            "wo": np.ascontiguousarray(w_out[cs, :]),
            "bq": np.ascontiguousarray(b_in[0:D][cs]),
            "bk": np.ascontiguousarray(b_in[D:2 * D][cs]),
            "bv": np.ascontiguousarray(b_in[2 * D:3 * D][cs]),
        })
    return in_maps


_NC_CACHE = None


def _get_nc():
    global _NC_CACHE
    if _NC_CACHE is None:
        _NC_CACHE = build_kernel()
    return _NC_CACHE


def run(x, w_in, b_in, w_out, b_out, trace=False, **spmd_kwargs):
    x = np.asarray(x, dtype=np.float32)
    w_in = np.asarray(w_in, dtype=np.float32)
    b_in = np.asarray(b_in, dtype=np.float32)
    w_out = np.asarray(w_out, dtype=np.float32)
    b_out = np.asarray(b_out, dtype=np.float32)

    nc = _get_nc()
    in_maps = _shard_inputs(x, w_in, b_in, w_out)
    res = run_bass_kernel_spmd(
        nc, in_maps, core_ids=list(range(NCORES)), trace=trace, **spmd_kwargs
    )
    out = np.zeros((B, L, D), dtype=np.float32)
    for c in range(NCORES):
        out[c // 4] += res.results[c]["out"]
    out += b_out[None, None, :]
    return out, res


def kernel(x, w_in, b_in, w_out, b_out):
    out, _ = run(x, w_in, b_in, w_out, b_out, trace=False)
    return out



# revision 7
# speedup vs baseline: 1.3585x; 1.3585x over previous
# Trainium2 Bass kernel for nn_CausalSelfAttention_13022340841799.
#
# Problem (hardcoded shapes): B=2, L=4096, D=512, 8 heads of dim 64.
#   qkv = x @ w_in + b_in; prefix-causal attention (PREFIX=1: tril mask with
#   column 0 disallowed for rows >= 1); out = attn_out @ w_out + b_out.
#
# Sharding: 8 cores = 2 batches x 4 head-pairs. Core c handles batch c//4 and
# heads {2*(c%4), 2*(c%4)+1}. Each core computes a partial [L, D] output
# (its heads' contribution through w_out); the host sums the 4 partials per
# batch and adds b_out.
#
# Device design (all bf16 compute, f32 PSUM):
#  - Host pre-transposes x to xT [128, 4, L] bf16 (no on-device transposes).
#  - qT/kT [feat, L] from wq/wk lhsT matmuls; v natural [L, feat] + ones col.
#  - S^T tiles [128 keys, 512 queries] per head computed as ROW-TILED pairs:
#    h0 on PE rows 0-63 (tile_position (0,0)), h1 on rows 64-127 ((64,0)) --
#    the two K=64 matmuls run concurrently in the array, out to 2 psum banks.
#  - Diagonal tiles only compute queries >= 128*d (the skipped region is
#    provably never read by PV). Their mask collapses to k <= q_local on the
#    first 128 query columns -> one shared [128,128] mask tile.
#  - Prefix (key-0) masking is folded into the exp bias (-80 at partition 0).
#  - exp is split between ACT (native Exp) and DVE (fast-exp bit trick:
#    P_bf16_bits = S*184.665 + 16251 as int16, bitcast to bf16).
#  - PV transposed: O^T_aug [65, 512] += v_aug_j^T @ P_j; row 64 accumulates
#    the softmax denominator via the ones column of v_aug.
#  - Normalize: DVE reciprocal of denom row -> PE ones-broadcast matmul to
#    spread 1/denom across partitions -> DVE mult -> O^T [64, 2, L] bf16.
#  - out partial = sum_h O_h^T.T @ wo_h in PSUM, DMA'd f32 PSUM->DRAM.
#  - Emission is software-pipelined per 512-row chunk so PE stays dense.

import numpy as np
import ml_dtypes

import concourse.bass as bass
import concourse.mybir as mybir
import concourse.tile as tile
from concourse import bacc
from concourse.bass_utils import run_bass_kernel_spmd

F32 = mybir.dt.float32
BF16 = mybir.dt.bfloat16
I16 = mybir.dt.int16

B, L, D = 2, 4096, 512
H, HD = 8, 64
HPC = 2                  # heads per core
CD = HPC * HD            # 128 per-core qkv feature columns
NCORES = 8
SCALE = 1.0 / 8.0        # 1/sqrt(64)
NRC = L // 512           # 8 row chunks
KC = D // 128            # 4 contraction chunks
MASK_NEG = -80.0         # pre-exp additive mask (exp(-80+s) ~ 0)
# fast-exp: bf16_bits(e^x) ~ x * (2^7/ln2) + (16256 - 5.46 + 0.5)
FE_A = 128.0 / float(np.log(2.0))
FE_B = 16251.04
LAG = 2                  # S-pair -> PV software pipeline distance


def _route_dve(r, j):
    """Which exp tiles go to DVE (fast-exp) vs ACT (native exp).

    DVE takes all diagonal pairs plus every 4th full pair; this puts
    ~36% of exp rows on DVE, balancing both engines' total load.
    """
    d = j - 4 * r
    if d >= 0:
        return True
    return (j % 4) == 1


def build_kernel():
    nc = bacc.Bacc(trn_type="TRN2", target_bir_lowering=False)

    xt_d = nc.declare_dram_parameter("xt", [128, KC, L], BF16, isOutput=False)
    wq_d = nc.declare_dram_parameter("wq", [128, KC, CD], BF16, isOutput=False)
    wk_d = nc.declare_dram_parameter("wk", [128, KC, CD], BF16, isOutput=False)
    wv_d = nc.declare_dram_parameter("wv", [128, KC, CD], BF16, isOutput=False)
    wo_d = nc.declare_dram_parameter("wo", [64, HPC, D], BF16, isOutput=False)
    bq_d = nc.declare_dram_parameter("bq", [CD, 1], F32, isOutput=False)
    bk_d = nc.declare_dram_parameter("bk", [CD, 1], F32, isOutput=False)
    bv_d = nc.declare_dram_parameter("bv", [1, CD], BF16, isOutput=False)
    out_d = nc.declare_dram_parameter("out", [L, D], BF16, isOutput=True)

    with tile.TileContext(nc) as tc:
        with (
            tc.tile_pool(name="const", bufs=1) as const,
            tc.tile_pool(name="ppool", bufs=4) as ppool,
            tc.tile_pool(name="work", bufs=2) as work,
            tc.tile_pool(name="psS", bufs=2, space="PSUM") as psS,
            tc.tile_pool(name="psPV", bufs=2, space="PSUM") as psPV,
            tc.tile_pool(name="psQK", bufs=1, space="PSUM") as psQK,
            tc.tile_pool(name="psVO", bufs=1, space="PSUM") as psVO,
        ):
            # ---- persistent SBUF tensors
            xT = const.tile([128, KC, L], BF16, name="xT")
            qT = const.tile([128, L], BF16, name="qT")
            kT = const.tile([128, L], BF16, name="kT")
            v_aug = [
                const.tile([128, L // 128, 65], BF16, name=f"vaug{h}")
                for h in range(HPC)
            ]
            OT = const.tile([64, HPC, L], BF16, name="OT")

            wq_s = const.tile([128, KC, CD], BF16, name="wq_s")
            wk_s = const.tile([128, KC, CD], BF16, name="wk_s")
            wv_s = const.tile([128, KC, CD], BF16, name="wv_s")
            wo_s = const.tile([64, HPC, D], BF16, name="wo_s")
            bq_s = const.tile([CD, 1], F32, name="bq_s")
            bk_s = const.tile([CD, 1], F32, name="bk_s")
            bv_s = const.tile([1, CD], BF16, name="bv_s")

            nc.sync.dma_start(wq_s, wq_d[:, :, :])
            nc.sync.dma_start(wk_s, wk_d[:, :, :])
            nc.sync.dma_start(wv_s, wv_d[:, :, :])
            nc.sync.dma_start(wo_s, wo_d[:, :, :])
            nc.sync.dma_start(bq_s, bq_d[:, :])
            nc.sync.dma_start(bk_s, bk_d[:, :])
            nc.sync.dma_start(bv_s, bv_d[:, :])
            for r in range(NRC):
                cs = slice(r * 512, (r + 1) * 512)
                nc.sync.dma_start(xT[:, :, cs], xt_d[:, :, cs])

            # ---- constants
            ones64 = const.tile([1, 64], BF16, name="ones64")
            nc.gpsimd.memset(ones64, 1.0)
            ones128 = const.tile([1, 128], BF16, name="ones128")
            nc.gpsimd.memset(ones128, 1.0)
            for h in range(HPC):
                nc.gpsimd.memset(v_aug[h][:, :, 64:65], 1.0)

            # causal mask tile: M[k, q] = 1 if k <= q else 0
            Mc = const.tile([128, 128], BF16, name="Mc")
            nc.gpsimd.memset(Mc, 1.0)
            nc.gpsimd.affine_select(
                out=Mc, in_=Mc, compare_op=mybir.AluOpType.is_ge, fill=0.0,
                base=0, channel_multiplier=-1, pattern=[[1, 128]],
            )
            # exp bias vectors (per-partition): key-0 prefix masking
            b0_act = const.tile([128, 1], F32, name="b0_act")
            nc.gpsimd.memset(b0_act, 0.0)
            nc.gpsimd.memset(b0_act[0:1, :], MASK_NEG)
            b_dve = const.tile([128, 1], F32, name="b_dve")
            nc.gpsimd.memset(b_dve, FE_B)
            b0_dve = const.tile([128, 1], F32, name="b0_dve")
            nc.gpsimd.memset(b0_dve, FE_B)
            nc.gpsimd.memset(b0_dve[0:1, :], FE_B + MASK_NEG * FE_A)

            # ---- per-chunk emission helpers
            def emit_qkv(r):
                cs = slice(r * 512, (r + 1) * 512)
                pq = psQK.tile([128, 512], F32, tag="qk")
                for d in range(KC):
                    nc.tensor.matmul(
                        pq, lhsT=wq_s[:, d, :], rhs=xT[:, d, cs],
                        start=(d == 0), stop=(d == KC - 1),
                    )
                nc.scalar.activation(
                    qT[:, cs], pq, mybir.ActivationFunctionType.Identity,
                    bias=bq_s, scale=SCALE,
                )
                pk = psQK.tile([128, 512], F32, tag="qk")
                for d in range(KC):
                    nc.tensor.matmul(
                        pk, lhsT=wk_s[:, d, :], rhs=xT[:, d, cs],
                        start=(d == 0), stop=(d == KC - 1),
                    )
                nc.scalar.activation(
                    kT[:, cs], pk, mybir.ActivationFunctionType.Identity,
                    bias=bk_s, scale=1.0,
                )
                pv = psVO.tile([128, 512], F32, tag="vo")
                for rb in range(4):
                    rs = slice((4 * r + rb) * 128, (4 * r + rb + 1) * 128)
                    ps = pv[:, rb * 128:(rb + 1) * 128]
                    for d in range(KC):
                        nc.tensor.matmul(
                            ps, lhsT=xT[:, d, rs], rhs=wv_s[:, d, :],
                            start=(d == 0), stop=False,
                        )
                    nc.tensor.matmul(
                        ps, lhsT=ones128, rhs=bv_s, start=False, stop=True,
                    )
                pvv = pv.rearrange("p (g c) -> p g c", c=128)
                for h in range(HPC):
                    nc.vector.tensor_copy(
                        v_aug[h][:, 4 * r:4 * r + 4, 0:64],
                        pvv[:, :, h * 64:(h + 1) * 64],
                    )

            def emit_S(r, j):
                d = j - 4 * r
                qoff = 128 * d if d >= 0 else 0
                w = 512 - qoff
                sp = psS.tile([128, 2, 512], F32, tag="sp")
                for h in range(HPC):
                    hs = slice(h * 64, (h + 1) * 64)
                    nc.tensor.matmul(
                        sp[:, h, 0:w],
                        lhsT=kT[hs, j * 128:(j + 1) * 128],
                        rhs=qT[hs, r * 512 + qoff:(r + 1) * 512],
                        start=True, stop=True,
                        tile_position=(64 * h, 0),
                    )
                # exp -> P bf16 (both heads in one instruction)
                pt = ppool.tile([128, 2, 512], BF16, tag="p")
                if _route_dve(r, j):
                    bias = b0_dve if j == 0 else b_dve
                    nc.vector.tensor_scalar(
                        out=pt.bitcast(I16)[:, :, 0:w], in0=sp[:, :, 0:w],
                        scalar1=FE_A, scalar2=bias,
                        op0=mybir.AluOpType.mult, op1=mybir.AluOpType.add,
                    )
                else:
                    bias = b0_act if j == 0 else 0.0
                    nc.scalar.activation(
                        pt[:, :, 0:w], sp[:, :, 0:w],
                        mybir.ActivationFunctionType.Exp,
                        bias=bias, scale=1.0,
                    )
                if d >= 0:
                    # diagonal: mask first 128 query cols with k<=q pattern
                    mb = bass.AP(
                        tensor=Mc.tensor, offset=Mc.offset,
                        ap=[list(Mc.ap[0]), [0, 2], [1, 128]],
                    )
                    nc.vector.tensor_tensor(
                        pt[:, :, 0:128], pt[:, :, 0:128], mb,
                        mybir.AluOpType.mult,
                    )
                if r == 0 and j == 0:
                    # query 0 attends only key 0: force P[0, 0] = 1
                    nc.vector.memset(pt[0:1, :, 0:1], 1.0)
                return pt

            def emit_PV(r, j, pv_ts, p_ts, nj):
                d = j - 4 * r
                qoff = 128 * d if d >= 0 else 0
                for h in range(HPC):
                    nc.tensor.matmul(
                        pv_ts[h][:, qoff:512],
                        lhsT=v_aug[h][:, j, :],
                        rhs=p_ts[j][:, h, 0:512 - qoff],
                        start=(j == 0), stop=(j == nj - 1),
                    )

            def emit_normalize(r, pv_ts):
                cs = slice(r * 512, (r + 1) * 512)
                rc = work.tile([1, 2, 512], BF16, tag="rc")
                with nc.allow_low_precision(reason="bf16 softmax denominators"):
                    for h in range(HPC):
                        nc.vector.reciprocal(rc[:, h, :], pv_ts[h][64:65, :])
                rb = psS.tile([128, 2, 512], F32, tag="sp")
                for h in range(HPC):
                    nc.tensor.matmul(
                        rb[0:64, h, :], lhsT=ones64, rhs=rc[:, h, :],
                        start=True, stop=True,
                    )
                rc_b = work.tile([64, 2, 512], BF16, tag="rc_b")
                nc.scalar.activation(
                    rc_b, rb[0:64, :, :], mybir.ActivationFunctionType.Copy)
                for h in range(HPC):
                    nc.vector.tensor_tensor(
                        OT[:, h, cs], pv_ts[h][0:64, :], rc_b[:, h, :],
                        mybir.AluOpType.mult,
                    )

            def emit_outproj(r, blk):
                bs = slice((4 * r + blk) * 128, (4 * r + blk + 1) * 128)
                op = psVO.tile([128, 512], F32, tag="vo")
                for h in range(HPC):
                    nc.tensor.matmul(
                        op, lhsT=OT[:, h, bs], rhs=wo_s[:, h, :],
                        start=(h == 0), stop=(h == HPC - 1),
                    )
                ost = work.tile([128, 512], BF16, tag="ost")
                if blk % 2 == 0:
                    nc.scalar.activation(
                        ost, op, mybir.ActivationFunctionType.Copy)
                else:
                    nc.vector.tensor_copy(ost, op)
                nc.sync.dma_start(out_d[bs, :], ost)

            # ---- main pipeline
            emit_qkv(0)
            for r in range(NRC):
                nj = 4 * r + 4
                pv_ts = [
                    psPV.tile([65, 512], F32, tag="pv", name=f"pv{h}")
                    for h in range(HPC)
                ]
                p_ts = {}
                for step in range(nj + LAG):
                    if step < nj:
                        p_ts[step] = emit_S(r, step)
                    if step == 1 and r + 1 < NRC:
                        emit_qkv(r + 1)
                    if r >= 1 and step in (1, 3, 5, 7):
                        emit_outproj(r - 1, (step - 1) // 2)
                    if step >= LAG:
                        j = step - LAG
                        emit_PV(r, j, pv_ts, p_ts, nj)
                        p_ts.pop(j)
                emit_normalize(r, pv_ts)
            for blk in range(4):
                emit_outproj(NRC - 1, blk)

    nc.finalize()
    return nc


def _shard_inputs(x, w_in, b_in, w_out):
    """Per-core input maps: core c -> batch c//4, heads pair c%4."""
    bf16 = ml_dtypes.bfloat16
    in_maps = []
    for c in range(NCORES):
        b = c // 4
        hp = c % 4
        cs = slice(hp * CD, hp * CD + CD)

        xt = np.ascontiguousarray(x[b].T)          # [D, L] f32
        xt = np.ascontiguousarray(
            xt.reshape(KC, 128, L).transpose(1, 0, 2), dtype=bf16)

        def wslice(lo):
            w = w_in[:, lo:lo + D][:, cs]           # [D, CD]
            return np.ascontiguousarray(
                w.reshape(KC, 128, CD).transpose(1, 0, 2), dtype=bf16)

        wo = w_out[cs, :]                           # [CD, D]
        wo = np.ascontiguousarray(
            wo.reshape(HPC, 64, D).transpose(1, 0, 2), dtype=bf16)

        in_maps.append({
            "xt": xt,
            "wq": wslice(0),
            "wk": wslice(D),
            "wv": wslice(2 * D),
            "wo": wo,
            "bq": np.ascontiguousarray(
                (b_in[0:D][cs] * SCALE).reshape(CD, 1), dtype=np.float32),
            "bk": np.ascontiguousarray(
                b_in[D:2 * D][cs].reshape(CD, 1), dtype=np.float32),
            "bv": np.ascontiguousarray(
                b_in[2 * D:3 * D][cs].reshape(1, CD), dtype=bf16),
        })
    return in_maps


_NC_CACHE = None


def _get_nc():
    global _NC_CACHE
    if _NC_CACHE is None:
        _NC_CACHE = build_kernel()
    return _NC_CACHE


def run(x, w_in, b_in, w_out, b_out, trace=False, **spmd_kwargs):
    x = np.asarray(x, dtype=np.float32)
    w_in = np.asarray(w_in, dtype=np.float32)
    b_in = np.asarray(b_in, dtype=np.float32)
    w_out = np.asarray(w_out, dtype=np.float32)
    b_out = np.asarray(b_out, dtype=np.float32)

    nc = _get_nc()
    in_maps = _shard_inputs(x, w_in, b_in, w_out)
    res = run_bass_kernel_spmd(
        nc, in_maps, core_ids=list(range(NCORES)), trace=trace, **spmd_kwargs
    )
    out = np.zeros((B, L, D), dtype=np.float32)
    for c in range(NCORES):
        out[c // 4] += np.asarray(res.results[c]["out"], dtype=np.float32)
    out += b_out[None, None, :]
    return out, res


def kernel(x, w_in, b_in, w_out, b_out):
    out, _ = run(x, w_in, b_in, w_out, b_out, trace=False)
    return out


# revision 12
# speedup vs baseline: 1.8876x; 1.3894x over previous
# Trainium2 Bass kernel for nn_CausalSelfAttention_13022340841799.
#
# Problem (hardcoded shapes): B=2, L=4096, D=512, 8 heads of dim 64.
#   qkv = x @ w_in + b_in; prefix-causal attention (PREFIX=1: tril mask with
#   column 0 disallowed for rows >= 1); out = attn_out @ w_out + b_out.
#
# Sharding: 8 cores = 2 batches x 4 head-pairs. Core c handles batch c//4 and
# heads {2*(c%4), 2*(c%4)+1}. Each core computes a partial [L, D] output
# (its heads' contribution through w_out); the host sums the 4 partials per
# batch and adds b_out.
#
# Device design (all bf16 compute, f32 PSUM):
#  - Host pre-transposes x to xT [128, 4, L] bf16 (no on-device transposes).
#  - qT/kT [feat, L] from wq/wk lhsT matmuls; v natural [L, feat] + ones col.
#  - S^T tiles [128 keys, 512 queries] per head computed as ROW-TILED pairs:
#    h0 on PE rows 0-63 (tile_position (0,0)), h1 on rows 64-127 ((64,0)) --
#    the two K=64 matmuls run concurrently in the array, out to 2 psum banks.
#  - Diagonal tiles only compute queries >= 128*d (the skipped region is
#    provably never read by PV). Their mask collapses to k <= q_local on the
#    first 128 query columns -> one shared [128,128] mask tile.
#  - Prefix (key-0) masking is folded into the exp bias (-80 at partition 0).
#  - exp is split between ACT (native Exp) and DVE (fast-exp bit trick:
#    P_bf16_bits = S*184.665 + 16251 as int16, bitcast to bf16).
#  - PV transposed: O^T_aug [65, 512] += v_aug_j^T @ P_j; row 64 accumulates
#    the softmax denominator via the ones column of v_aug.
#  - Normalize: DVE reciprocal of denom row -> PE ones-broadcast matmul to
#    spread 1/denom across partitions -> DVE mult -> O^T [64, 2, L] bf16.
#  - out partial = sum_h O_h^T.T @ wo_h in PSUM, DMA'd f32 PSUM->DRAM.
#  - Emission is software-pipelined per 512-row chunk so PE stays dense.

import numpy as np
import ml_dtypes

import concourse.bass as bass
import concourse.mybir as mybir
import concourse.tile as tile
from concourse import bacc
from concourse.bass_utils import run_bass_kernel_spmd

F32 = mybir.dt.float32
BF16 = mybir.dt.bfloat16
I16 = mybir.dt.int16

B, L, D = 2, 4096, 512
H, HD = 8, 64
HPC = 2                  # heads per core
CD = HPC * HD            # 128 per-core qkv feature columns
NCORES = 8
SCALE = 1.0 / 8.0        # 1/sqrt(64)
NRC = L // 512           # 8 row chunks
KC = D // 128            # 4 contraction chunks
MASK_NEG = -80.0         # pre-exp additive mask (exp(-80+s) ~ 0)
# fast-exp: bf16_bits(e^x) ~ x * (2^7/ln2) + (16256 - 5.46 + 0.5)
FE_A = 128.0 / float(np.log(2.0))
FE_B = 16251.04
LAG = 2                  # S-pair -> PV software pipeline distance


def _route_dve(r, j):
    """Which exp tiles go to DVE (fast-exp) vs ACT (native exp).

    DVE takes all diagonal pairs plus every 4th full pair; this puts
    ~36% of exp rows on DVE, balancing both engines' total load.
    """
    d = j - 4 * r
    if d >= 0:
        return True
    return (j % 4) == 1


def build_kernel(dbg=False):
    nc = bacc.Bacc(trn_type="TRN2", target_bir_lowering=False)

    xt_d = nc.declare_dram_parameter("xt", [128, KC, L], BF16, isOutput=False)
    wq_d = nc.declare_dram_parameter("wq", [128, KC, CD], BF16, isOutput=False)
    wk_d = nc.declare_dram_parameter("wk", [128, KC, CD], BF16, isOutput=False)
    wv_d = nc.declare_dram_parameter("wv", [128, KC, CD], BF16, isOutput=False)
    wo_d = nc.declare_dram_parameter("wo", [64, HPC, D], BF16, isOutput=False)
    bq_d = nc.declare_dram_parameter("bq", [CD, 1], F32, isOutput=False)
    bk_d = nc.declare_dram_parameter("bk", [CD, 1], F32, isOutput=False)
    bv_d = nc.declare_dram_parameter("bv", [1, CD], BF16, isOutput=False)
    out_d = nc.declare_dram_parameter("out", [L, D], BF16, isOutput=True)
    if dbg:
        otd = nc.declare_dram_parameter("ot_dbg", [64, HPC, L], BF16, isOutput=True)
        ord_ = nc.declare_dram_parameter("or_dbg", [64, HPC, L], BF16, isOutput=True)
        rcd = nc.declare_dram_parameter("rc_dbg", [64, HPC, L], F32, isOutput=True)

    with tile.TileContext(nc) as tc:
        with (
            tc.tile_pool(name="const", bufs=1) as const,
            tc.tile_pool(name="ppool", bufs=4) as ppool,
            tc.tile_pool(name="work", bufs=2) as work,
            tc.tile_pool(name="psS", bufs=2, space="PSUM") as psS,
            tc.tile_pool(name="psPV", bufs=2, space="PSUM") as psPV,
            tc.tile_pool(name="psQK", bufs=1, space="PSUM") as psQK,
            tc.tile_pool(name="psVO", bufs=1, space="PSUM") as psVO,
            tc.tile_pool(name="dramp", bufs=2, space="DRAM") as dramp,
        ):
            # ---- persistent SBUF tensors
            xT = const.tile([128, KC, L], BF16, name="xT")
            qT = const.tile([128, L], BF16, name="qT")
            kT = const.tile([128, L], BF16, name="kT")
            v_aug = [
                const.tile([128, L // 128, 65], BF16, name=f"vaug{h}")
                for h in range(HPC)
            ]
            OT = const.tile([64, HPC, L], BF16, name="OT")

            wq_s = const.tile([128, KC, CD], BF16, name="wq_s")
            wk_s = const.tile([128, KC, CD], BF16, name="wk_s")
            wv_s = const.tile([128, KC, CD], BF16, name="wv_s")
            wo_s = const.tile([64, HPC, D], BF16, name="wo_s")
            bq_s = const.tile([CD, 1], F32, name="bq_s")
            bk_s = const.tile([CD, 1], F32, name="bk_s")
            bv_s = const.tile([1, CD], BF16, name="bv_s")

            nc.sync.dma_start(wq_s, wq_d[:, :, :])
            nc.sync.dma_start(wk_s, wk_d[:, :, :])
            nc.sync.dma_start(wv_s, wv_d[:, :, :])
            nc.sync.dma_start(wo_s, wo_d[:, :, :])
            nc.sync.dma_start(bq_s, bq_d[:, :])
            nc.sync.dma_start(bk_s, bk_d[:, :])
            nc.sync.dma_start(bv_s, bv_d[:, :])
            for r in range(NRC):
                cs = slice(r * 512, (r + 1) * 512)
                nc.sync.dma_start(xT[:, :, cs], xt_d[:, :, cs])

            # ---- constants
            ones128 = const.tile([1, 128], BF16, name="ones128")
            nc.gpsimd.memset(ones128, 1.0)
            for h in range(HPC):
                nc.gpsimd.memset(v_aug[h][:, :, 64:65], 1.0)

            # causal mask tile: M[k, q] = 1 if k <= q else 0
            Mc = const.tile([128, 128], BF16, name="Mc")
            nc.gpsimd.memset(Mc, 1.0)
            nc.gpsimd.affine_select(
                out=Mc, in_=Mc, compare_op=mybir.AluOpType.is_ge, fill=0.0,
                base=0, channel_multiplier=-1, pattern=[[1, 128]],
            )
            # exp bias vectors (per-partition): key-0 prefix masking
            b0_act = const.tile([128, 1], F32, name="b0_act")
            nc.gpsimd.memset(b0_act, 0.0)
            nc.gpsimd.memset(b0_act[0:1, :], MASK_NEG)
            b_dve = const.tile([128, 1], F32, name="b_dve")
            nc.gpsimd.memset(b_dve, FE_B)
            b0_dve = const.tile([128, 1], F32, name="b0_dve")
            nc.gpsimd.memset(b0_dve, FE_B)
            nc.gpsimd.memset(b0_dve[0:1, :], FE_B + MASK_NEG * FE_A)

            # ---- per-chunk emission helpers
            def emit_qkv(r):
                cs = slice(r * 512, (r + 1) * 512)
                pq = psQK.tile([128, 512], F32, tag="qk")
                for d in range(KC):
                    nc.tensor.matmul(
                        pq, lhsT=wq_s[:, d, :], rhs=xT[:, d, cs],
                        start=(d == 0), stop=(d == KC - 1),
                    )
                nc.scalar.activation(
                    qT[:, cs], pq, mybir.ActivationFunctionType.Identity,
                    bias=bq_s, scale=SCALE,
                )
                pk = psQK.tile([128, 512], F32, tag="qk")
                for d in range(KC):
                    nc.tensor.matmul(
                        pk, lhsT=wk_s[:, d, :], rhs=xT[:, d, cs],
                        start=(d == 0), stop=(d == KC - 1),
                    )
                nc.scalar.activation(
                    kT[:, cs], pk, mybir.ActivationFunctionType.Identity,
                    bias=bk_s, scale=1.0,
                )
                pv = psVO.tile([128, 512], F32, tag="vo")
                for rb in range(4):
                    rs = slice((4 * r + rb) * 128, (4 * r + rb + 1) * 128)
                    ps = pv[:, rb * 128:(rb + 1) * 128]
                    for d in range(KC):
                        nc.tensor.matmul(
                            ps, lhsT=xT[:, d, rs], rhs=wv_s[:, d, :],
                            start=(d == 0), stop=False,
                        )
                    nc.tensor.matmul(
                        ps, lhsT=ones128, rhs=bv_s, start=False, stop=True,
                    )
                pvv = pv.rearrange("p (g c) -> p g c", c=128)
                for h in range(HPC):
                    nc.vector.tensor_copy(
                        v_aug[h][:, 4 * r:4 * r + 4, 0:64],
                        pvv[:, :, h * 64:(h + 1) * 64],
                    )

            def emit_S(r, j):
                d = j - 4 * r
                qoff = 128 * d if d >= 0 else 0
                w = 512 - qoff
                sp = psS.tile([128, 2, 512], F32, tag="sp")
                for h in range(HPC):
                    hs = slice(h * 64, (h + 1) * 64)
                    nc.tensor.matmul(
                        sp[:, h, 0:w],
                        lhsT=kT[hs, j * 128:(j + 1) * 128],
                        rhs=qT[hs, r * 512 + qoff:(r + 1) * 512],
                        start=True, stop=True,
                        tile_position=(64 * h, 0),
                    )
                # exp -> P bf16 (both heads in one instruction)
                pt = ppool.tile([128, 2, 512], BF16, tag="p")
                if _route_dve(r, j):
                    bias = b0_dve if j == 0 else b_dve
                    nc.vector.tensor_scalar(
                        out=pt.bitcast(I16)[:, :, 0:w], in0=sp[:, :, 0:w],
                        scalar1=FE_A, scalar2=bias,
                        op0=mybir.AluOpType.mult, op1=mybir.AluOpType.add,
                    )
                else:
                    bias = b0_act if j == 0 else 0.0
                    nc.scalar.activation(
                        pt[:, :, 0:w], sp[:, :, 0:w],
                        mybir.ActivationFunctionType.Exp,
                        bias=bias, scale=1.0,
                    )
                if d >= 0:
                    # diagonal: mask first 128 query cols with k<=q pattern
                    mb = bass.AP(
                        tensor=Mc.tensor, offset=Mc.offset,
                        ap=[list(Mc.ap[0]), [0, 2], [1, 128]],
                    )
                    nc.vector.tensor_tensor(
                        pt[:, :, 0:128], pt[:, :, 0:128], mb,
                        mybir.AluOpType.mult,
                    )
                if r == 0 and j == 0:
                    # query 0 attends only key 0: force P[0, 0] = 1
                    nc.vector.memset(pt[0:1, :, 0:1], 1.0)
                return pt

            def emit_PV(r, j, pv_ts, p_ts, nj):
                d = j - 4 * r
                qoff = 128 * d if d >= 0 else 0
                for h in range(HPC):
                    nc.tensor.matmul(
                        pv_ts[h][:, qoff:512],
                        lhsT=v_aug[h][:, j, :],
                        rhs=p_ts[j][:, h, 0:512 - qoff],
                        start=(j == 0), stop=(j == nj - 1),
                    )

            def emit_recip(r, pv_ts):
                # Evacuate O_raw psum->SBUF and start the 1/denominator
                # broadcast (pipelined DRAM bounce, consumed ~5us later).
                # Must be emitted before the next chunk's PV claims so the
                # psum WAR deps see every reader.
                dn = work.tile([65, 2, 512], F32, tag="dn")
                for h in range(HPC):
                    # psum reads must stay partition-aligned (engines cannot
                    # shift partitions)
                    nc.scalar.activation(
                        dn[64:65, h, :], pv_ts[h][64:65, :],
                        mybir.ActivationFunctionType.Copy)
                o_raw = work.tile([64, 2, 512], BF16, tag="o_raw")
                for h in range(HPC):
                    nc.scalar.activation(
                        o_raw[:, h, :], pv_ts[h][0:64, :],
                        mybir.ActivationFunctionType.Copy)
                # broadcast raw denominators to partitions 0-63 via DRAM
                # bounce, then invert there (reciprocal_approx_fast only
                # works at base partition 0)
                scr = dramp.tile([1, 2, 512], F32, tag="scr")
                nc.sync.dma_start(out=scr, in_=dn[64:65, :, :])
                dnb = work.tile([64, 2, 512], F32, tag="dnb")
                s = scr[0:1, :, :]
                src_b = bass.AP(
                    tensor=s.tensor, offset=s.offset,
                    ap=[[0, 64]] + [list(p) for p in s.ap[1:]],
                )
                nc.sync.dma_start(out=dnb, in_=src_b)
                rc_b = work.tile([64, 2, 512], F32, tag="rc_b")
                nc.vector.reciprocal_approx_fast(out=rc_b, in_=dnb)
                return o_raw, rc_b

            def emit_norm(r, o_raw, rc_b):
                cs = slice(r * 512, (r + 1) * 512)
                for h in range(HPC):
                    nc.vector.tensor_tensor(
                        OT[:, h, cs], o_raw[:, h, :], rc_b[:, h, :],
                        mybir.AluOpType.mult,
                    )
                if dbg:
                    nc.sync.dma_start(ord_[:, :, cs], o_raw)
                    nc.sync.dma_start(rcd[:, :, cs], rc_b)

            def emit_outproj(r, blk):
                bs = slice((4 * r + blk) * 128, (4 * r + blk + 1) * 128)
                op = psVO.tile([128, 512], F32, tag="vo")
                for h in range(HPC):
                    nc.tensor.matmul(
                        op, lhsT=OT[:, h, bs], rhs=wo_s[:, h, :],
                        start=(h == 0), stop=(h == HPC - 1),
                    )
                ost = work.tile([128, 512], BF16, tag="ost")
                if blk % 2 == 0:
                    nc.scalar.activation(
                        ost, op, mybir.ActivationFunctionType.Copy)
                else:
                    nc.vector.tensor_copy(ost, op)
                nc.sync.dma_start(out_d[bs, :], ost)

            # ---- main pipeline
            emit_qkv(0)
            prev = None           # (r-1)'s pv tiles, for pipelined normalize
            for r in range(NRC):
                nj = 4 * r + 4
                nstep = nj + LAG
                op_stride = 1 if nstep < 13 else 2
                op_slots = [5 + i * op_stride for i in range(4)]
                pv_ts = [
                    psPV.tile([65, 512], F32, tag="pv", name=f"pv{h}")
                    for h in range(HPC)
                ]
                p_ts = {}
                for step in range(nstep):
                    if step < nj:
                        p_ts[step] = emit_S(r, step)
                    if step == 1 and r + 1 < NRC:
                        emit_qkv(r + 1)
                    if step == 3 and prev is not None:
                        emit_norm(r - 1, *prev)
                    if prev is not None and step in op_slots:
                        emit_outproj(r - 1, op_slots.index(step))
                    if step >= LAG:
                        j = step - LAG
                        emit_PV(r, j, pv_ts, p_ts, nj)
                        p_ts.pop(j)
                prev = emit_recip(r, pv_ts)
            emit_norm(NRC - 1, *prev)
            for blk in range(4):
                emit_outproj(NRC - 1, blk)
            if dbg:
                nc.sync.dma_start(otd[:, :, :], OT)

    nc.finalize()
    return nc


def _shard_inputs(x, w_in, b_in, w_out):
    """Per-core input maps: core c -> batch c//4, heads pair c%4."""
    bf16 = ml_dtypes.bfloat16
    in_maps = []
    for c in range(NCORES):
        b = c // 4
        hp = c % 4
        cs = slice(hp * CD, hp * CD + CD)

        xt = np.ascontiguousarray(x[b].T)          # [D, L] f32
        xt = np.ascontiguousarray(
            xt.reshape(KC, 128, L).transpose(1, 0, 2), dtype=bf16)

        def wslice(lo):
            w = w_in[:, lo:lo + D][:, cs]           # [D, CD]
            return np.ascontiguousarray(
                w.reshape(KC, 128, CD).transpose(1, 0, 2), dtype=bf16)

        wo = w_out[cs, :]                           # [CD, D]
        wo = np.ascontiguousarray(
            wo.reshape(HPC, 64, D).transpose(1, 0, 2), dtype=bf16)

        in_maps.append({
            "xt": xt,
            "wq": wslice(0),
            "wk": wslice(D),
            "wv": wslice(2 * D),
            "wo": wo,
            "bq": np.ascontiguousarray(
                (b_in[0:D][cs] * SCALE).reshape(CD, 1), dtype=np.float32),
            "bk": np.ascontiguousarray(
                b_in[D:2 * D][cs].reshape(CD, 1), dtype=np.float32),
            "bv": np.ascontiguousarray(
                b_in[2 * D:3 * D][cs].reshape(1, CD), dtype=bf16),
        })
    return in_maps


_NC_CACHE = None


def _get_nc():
    global _NC_CACHE
    if _NC_CACHE is None:
        _NC_CACHE = build_kernel()
    return _NC_CACHE


def run(x, w_in, b_in, w_out, b_out, trace=False, **spmd_kwargs):
    x = np.asarray(x, dtype=np.float32)
    w_in = np.asarray(w_in, dtype=np.float32)
    b_in = np.asarray(b_in, dtype=np.float32)
    w_out = np.asarray(w_out, dtype=np.float32)
    b_out = np.asarray(b_out, dtype=np.float32)

    nc = _get_nc()
    in_maps = _shard_inputs(x, w_in, b_in, w_out)
    res = run_bass_kernel_spmd(
        nc, in_maps, core_ids=list(range(NCORES)), trace=trace, **spmd_kwargs
    )
    out = np.zeros((B, L, D), dtype=np.float32)
    for c in range(NCORES):
        out[c // 4] += np.asarray(res.results[c]["out"], dtype=np.float32)
    out += b_out[None, None, :]
    return out, res


def kernel(x, w_in, b_in, w_out, b_out):
    out, _ = run(x, w_in, b_in, w_out, b_out, trace=False)
    return out
